# revision 12
# baseline (speedup 1.0000x reference)
"""Per-sample dynamic 3x3 conv (DCConv2d) on 8 Trainium2 NeuronCores.

Strategy: pure data parallel. Each core gets B_LOCAL=16 samples, processed in
4 groups of 4 samples packed onto the PE array as four diagonal 32x32 blocks
(block-diagonal bf16 stationary). The 3x3 conv is 9 PSUM-accumulated matmuls
(one per tap) over host-pre-padded bf16 images resident in SBUF as
[128 partitions = (sample, channel), 130, 130]. Host pre-padding makes every
input DMA a contiguous 33.8KB-per-partition transfer (full HBM bandwidth) and
eliminates on-device border memsets. Per-sample weights (inputs_se @ bank)
are generated on-device in fp32 via ACT per-partition scale + DVE adds, then
cast to bf16 into the block-diagonal stationary tile. The epilogue drains
PSUM to bf16 (bias fused, alternating ACT/DVE) and stores two 4-row chunks
per DMA so each store writes 2KB contiguous per partition.
"""

import numpy as np
import ml_dtypes

import concourse.bass as bass
import concourse.mybir as mybir
import concourse.tile as tile
from concourse.bass_utils import run_bass_kernel_spmd

N_CORES = 8
B, C, H, W = 128, 32, 128, 128
O = 32
NUM = 8
KK = 3
B_LOCAL = B // N_CORES          # 16
GROUP = 4                       # samples packed per PE pass
N_GROUPS = B_LOCAL // GROUP     # 4
HP, WP = H + 2, W + 2           # zero-padded image dims (host-padded)
ROWS_PER_CHUNK = 4              # output rows per matmul chunk (N = 4*128 = 512)
N_CHUNKS = H // ROWS_PER_CHUNK  # 32
NTAPS = KK * KK                 # 9
WBD_BUFS = 3                    # wfin pool depth; wbd slots memset once each

F32 = mybir.dt.float32
BF16 = mybir.dt.bfloat16
NP_BF16 = ml_dtypes.bfloat16


def _split_multiwait_insts(nc):
    """This walrus build encodes at most one sync-wait per instruction; Tile's
    tail drain carries one wait per hardware proc used. Split the extras into
    single-wait NOPs on the same engine, inserted just before."""
    for f in nc.m.functions:
        for blk in f.blocks:
            new_list = []
            changed = False
            for inst in blk.instructions:
                si = inst.sync_info
                if si is not None and len(si.on_wait) > 1:
                    waits = list(si.on_wait)
                    for j, w in enumerate(waits[:-1]):
                        new_list.append(
                            mybir.InstNoOp(
                                name=f"{inst.name}-ws-{j}",
                                engine=inst.engine,
                                ins=[],
                                outs=[],
                                sync_info=mybir.SyncInfo(on_wait=[w], on_update=[]),
                            )
                        )
                    inst.sync_info = mybir.SyncInfo(
                        on_wait=[waits[-1]], on_update=list(si.on_update)
                    )
                    changed = True
                new_list.append(inst)
            if changed:
                blk.instructions = new_list


def build_program(b_local=B_LOCAL, split_waits=True, reps=1):
    n_groups = b_local // GROUP
    nc = bass.Bass(
        "TRN2",
        target_bir_lowering=False,
        debug=False,
        num_devices=N_CORES,
        enable_partition_id=False,
    )
    # host-pre-padded bf16 images, grouped so each load engages all 128
    # partitions: x[g, (s c), 130, 130]
    x_d = nc.dram_tensor(
        "x", [n_groups, GROUP * C, HP, WP], BF16, kind="ExternalInput"
    ).ap()
    # wb: host-permuted weight bank replicated over the 4 sample slots,
    # wb[32s+i, n, t*32+o] = weight[o*288+i*9+t, n]
    wb_d = nc.dram_tensor(
        "wb", [GROUP * C, NUM, NTAPS * O], F32, kind="ExternalInput"
    ).ap()
    # se: host-replicated, se[32*s+i, g, n] = inputs_se[core_base + g*4+s, n]
    se_d = nc.dram_tensor("se", [128, n_groups, NUM], F32, kind="ExternalInput").ap()
    # bias replicated 4x across partition groups: [128, 1]
    bias_d = nc.dram_tensor("bias", [128, 1], F32, kind="ExternalInput").ap()
    y_d = nc.dram_tensor("y", [b_local, O, H, W], BF16, kind="ExternalOutput").ap()

    with tile.TileContext(nc) as tc:
        with (
            tc.tile_pool(name="xpool", bufs=3) as xpool,
            tc.tile_pool(name="wbpool", bufs=1) as wbpool,
            tc.tile_pool(name="wfin", bufs=WBD_BUFS) as wfin_pool,
            tc.tile_pool(name="wtmp", bufs=6) as wtmp_pool,
            tc.tile_pool(name="sepool", bufs=1) as sepool,
            tc.tile_pool(name="outp", bufs=6) as outp,
            tc.tile_pool(name="psum", bufs=6, space="PSUM") as psump,
        ):
            # --- one-time loads (on scalar queue so x loads go first) -----
            wbsb = wbpool.tile([128, NUM, NTAPS, O], F32)
            nc.scalar.dma_start(out=wbsb[:], in_=wb_d[:])
            se_sb = sepool.tile([128, n_groups, NUM], F32)
            nc.scalar.dma_start(out=se_sb[:], in_=se_d[:])
            bias_sb = sepool.tile([128, 1], F32)
            nc.scalar.dma_start(out=bias_sb[:], in_=bias_d[:])

            from contextlib import nullcontext

            rep_loop = tc.For_i(0, reps, 1) if reps > 1 else nullcontext()
            with rep_loop:
                _emit_body(nc, tc, n_groups, x_d, y_d, wbsb, se_sb, bias_sb,
                           xpool, wfin_pool, wtmp_pool, outp, psump)

    if split_waits:
        _split_multiwait_insts(nc)
    return nc


def _emit_body(nc, tc, n_groups, x_d, y_d, wbsb, se_sb, bias_sb,
               xpool, wfin_pool, wtmp_pool, outp, psump):
    for g in range(n_groups):
        # --- load pre-padded group images: one 128-partition DMA ------
        xt = xpool.tile([128, HP, WP], BF16)
        nc.sync.dma_start(out=xt[:], in_=x_d[g])

        # --- per-sample weight generation -----------------------------
        # wfin[(s,i), t, o] = sum_n se[(s,i),g,n] * wbsb[(s,i),n,t,o]
        wfin = wfin_pool.tile([128, NTAPS, O], F32)
        tmps = []
        for n in range(1, NUM):
            t_ = wtmp_pool.tile([128, NTAPS, O], F32, tag="wtmp")
            nc.scalar.activation(
                t_[:],
                wbsb[:, n],
                mybir.ActivationFunctionType.Identity,
                scale=se_sb[:, g, n : n + 1],
            )
            tmps.append(t_)
        nc.scalar.activation(
            wfin[:],
            wbsb[:, 0],
            mybir.ActivationFunctionType.Identity,
            scale=se_sb[:, g, 0:1],
        )
        for t_ in tmps:
            nc.vector.tensor_add(wfin[:], wfin[:], t_[:])
        # block-diagonal bf16 stationary: wbd[(s,i), t, 32s+o] = wfin,
        # zeros elsewhere (zeroed once per slot; diagonal blocks
        # overwritten each group).
        wbd = wfin_pool.tile([128, NTAPS, 128], BF16, tag="wbd")
        if g < WBD_BUFS:
            nc.gpsimd.memset(wbd[:], 0.0)
        for s in range(GROUP):
            nc.vector.tensor_copy(
                wbd[32 * s : 32 * (s + 1), :, 32 * s : 32 * (s + 1)],
                wfin[32 * s : 32 * (s + 1)],
            )

        # --- conv: chunks of 4 output rows, stores pack 2 chunks ------
        ob = None
        for ci in range(N_CHUNKS):
            h0 = ci * ROWS_PER_CHUNK
            ps = psump.tile([128, ROWS_PER_CHUNK * W], F32)
            for tap in range(NTAPS):
                kh, kw = divmod(tap, KK)
                nc.tensor.matmul(
                    ps[:],
                    wbd[:, tap, :],
                    xt[:, h0 + kh : h0 + kh + ROWS_PER_CHUNK, kw : kw + W],
                    start=(tap == 0),
                    stop=(tap == NTAPS - 1),
                )
            # drain PSUM -> bf16 SBUF with fused bias, alternating the
            # engine so neither queue serializes the epilogue; one DMA per
            # two chunks so each store writes 2KB contiguous per partition.
            if ci % 2 == 0:
                ob = outp.tile([128, 2, ROWS_PER_CHUNK * W], BF16)
                nc.scalar.activation(
                    ob[:, 0],
                    ps[:],
                    mybir.ActivationFunctionType.Identity,
                    bias=bias_sb[:, 0:1],
                )
            else:
                nc.vector.tensor_scalar_add(ob[:, 1], ps[:], bias_sb[:, 0:1])
                store_eng = nc.scalar if (ci // 2) % 2 == 0 else nc.sync
                store_eng.dma_start(
                    out=y_d[
                        g * GROUP : (g + 1) * GROUP,
                        :,
                        h0 - ROWS_PER_CHUNK : h0 + ROWS_PER_CHUNK,
                        :,
                    ],
                    in_=ob[:],
                )


def _host_prep(inputs, inputs_se, weight, bias):
    """Shard + relayout the inputs for the 8 per-core programs."""
    inputs_se = np.asarray(inputs_se, dtype=np.float32)
    weight = np.asarray(weight, dtype=np.float32)
    bias = np.asarray(bias, dtype=np.float32)

    # pre-padded bf16 images
    x_pad = np.zeros((B, C, HP, WP), dtype=NP_BF16)
    x_pad[:, :, 1 : H + 1, 1 : W + 1] = np.asarray(inputs, dtype=np.float32)

    # wb[32s+i, n, t*32+o] = weight[o*288 + i*9 + t, n], tiled over s
    wb = weight.reshape(O, C, NTAPS, NUM)          # [o, i, t, n]
    wb = wb.transpose(1, 3, 2, 0).reshape(C, NUM, NTAPS * O)
    wb = np.ascontiguousarray(np.tile(wb, (GROUP, 1, 1)))  # [128, 8, 288]
    bias_rep = np.ascontiguousarray(np.tile(bias, GROUP)[:, None])  # [128, 1]

    in_maps = []
    for core in range(N_CORES):
        b0 = core * B_LOCAL
        se_loc = inputs_se[b0 : b0 + B_LOCAL]      # [16, 8]
        # se[32*s+i, g, n] = se_loc[g*4+s, n]
        se_exp = np.repeat(
            se_loc.reshape(N_GROUPS, GROUP, NUM).transpose(1, 0, 2), 32, axis=0
        )  # [4*32, g, n] with (s, i) partition order
        in_maps.append(
            {
                "x": x_pad[b0 : b0 + B_LOCAL].reshape(
                    N_GROUPS, GROUP * C, HP, WP
                ),
                "wb": wb,
                "se": np.ascontiguousarray(se_exp, dtype=np.float32),
                "bias": bias_rep,
            }
        )
    return in_maps


_NC_CACHE = {}


def kernel(inputs, inputs_se, weight, bias):
    if "nc" not in _NC_CACHE:
        _NC_CACHE["nc"] = build_program()
    nc = _NC_CACHE["nc"]
    in_maps = _host_prep(inputs, inputs_se, weight, bias)
    res = run_bass_kernel_spmd(nc, in_maps, list(range(N_CORES)))
    out = np.concatenate(
        [np.asarray(res.results[i]["y"]) for i in range(N_CORES)], axis=0
    )
    return out.astype(np.float32)


# revision 15
# speedup vs baseline: 1.0764x; 1.0764x over previous
"""Per-sample dynamic 3x3 conv (DCConv2d) on 8 Trainium2 NeuronCores.

Strategy: pure data parallel. Each core gets B_LOCAL=16 samples, processed in
4 groups of 4 samples packed onto the PE array as four diagonal 32x32 blocks
(block-diagonal bf16 stationary). The 3x3 conv is 9 PSUM-accumulated matmuls
(one per tap) over host-pre-padded bf16 images resident in SBUF as
[128 partitions = (sample, channel), 130, 130]. Host pre-padding makes every
input DMA a contiguous 33.8KB-per-partition transfer (full HBM bandwidth) and
eliminates on-device border memsets. Per-sample weights (inputs_se @ bank)
are generated on-device in fp32 via ACT per-partition scale + DVE adds, then
cast to bf16 into the block-diagonal stationary tile. The epilogue drains
PSUM to bf16 (bias fused, alternating ACT/DVE) and stores two 4-row chunks
per DMA so each store writes 2KB contiguous per partition.
"""

import numpy as np
import ml_dtypes

import concourse.bass as bass
import concourse.mybir as mybir
import concourse.tile as tile
from concourse.bass_utils import run_bass_kernel_spmd

N_CORES = 8
B, C, H, W = 128, 32, 128, 128
O = 32
NUM = 8
KK = 3
B_LOCAL = B // N_CORES          # 16
GROUP = 4                       # samples packed per PE pass
N_GROUPS = B_LOCAL // GROUP     # 4
HP, WP = H + 2, W + 2           # zero-padded image dims (host-padded)
ROWS_PER_CHUNK = 4              # output rows per matmul chunk (N = 4*128 = 512)
N_CHUNKS = H // ROWS_PER_CHUNK  # 32
NTAPS = KK * KK                 # 9
WBD_BUFS = 3                    # wfin pool depth; wbd slots memset once each

F32 = mybir.dt.float32
BF16 = mybir.dt.bfloat16
NP_BF16 = ml_dtypes.bfloat16


def _split_multiwait_insts(nc):
    """This walrus build encodes at most one sync-wait per instruction; Tile's
    tail drain carries one wait per hardware proc used. Split the extras into
    single-wait NOPs on the same engine, inserted just before."""
    for f in nc.m.functions:
        for blk in f.blocks:
            new_list = []
            changed = False
            for inst in blk.instructions:
                si = inst.sync_info
                if si is not None and len(si.on_wait) > 1:
                    waits = list(si.on_wait)
                    for j, w in enumerate(waits[:-1]):
                        new_list.append(
                            mybir.InstNoOp(
                                name=f"{inst.name}-ws-{j}",
                                engine=inst.engine,
                                ins=[],
                                outs=[],
                                sync_info=mybir.SyncInfo(on_wait=[w], on_update=[]),
                            )
                        )
                    inst.sync_info = mybir.SyncInfo(
                        on_wait=[waits[-1]], on_update=list(si.on_update)
                    )
                    changed = True
                new_list.append(inst)
            if changed:
                blk.instructions = new_list


def build_program(b_local=B_LOCAL, split_waits=True, reps=1):
    n_groups = b_local // GROUP
    nc = bass.Bass(
        "TRN2",
        target_bir_lowering=False,
        debug=False,
        num_devices=N_CORES,
        enable_partition_id=False,
    )
    # host-pre-padded bf16 images, grouped so each load engages all 128
    # partitions: x[g, (s c), 130, 130]
    x_d = nc.dram_tensor(
        "x", [n_groups, GROUP * C, HP, WP], BF16, kind="ExternalInput"
    ).ap()
    # wb: host-permuted weight bank replicated over the 4 sample slots,
    # wb[32s+i, n, t*32+o] = weight[o*288+i*9+t, n]
    wb_d = nc.dram_tensor(
        "wb", [GROUP * C, NUM, NTAPS * O], F32, kind="ExternalInput"
    ).ap()
    # se: host-replicated, se[32*s+i, g, n] = inputs_se[core_base + g*4+s, n]
    se_d = nc.dram_tensor("se", [128, n_groups, NUM], F32, kind="ExternalInput").ap()
    # bias replicated 4x across partition groups: [128, 1]
    bias_d = nc.dram_tensor("bias", [128, 1], F32, kind="ExternalInput").ap()
    y_d = nc.dram_tensor("y", [b_local, O, H, W], BF16, kind="ExternalOutput").ap()

    with tile.TileContext(nc) as tc:
        with (
            tc.tile_pool(name="xpool", bufs=3) as xpool,
            tc.tile_pool(name="wbpool", bufs=1) as wbpool,
            tc.tile_pool(name="wfin", bufs=WBD_BUFS) as wfin_pool,
            tc.tile_pool(name="wtmp", bufs=6) as wtmp_pool,
            tc.tile_pool(name="sepool", bufs=1) as sepool,
            tc.tile_pool(name="outp", bufs=6) as outp,
            tc.tile_pool(name="psum", bufs=8, space="PSUM") as psump,
        ):
            # --- one-time loads (on scalar queue so x loads go first) -----
            wbsb = wbpool.tile([128, NUM, NTAPS, O], F32)
            nc.scalar.dma_start(out=wbsb[:], in_=wb_d[:])
            se_sb = sepool.tile([128, n_groups, NUM], F32)
            nc.scalar.dma_start(out=se_sb[:], in_=se_d[:])
            bias_sb = sepool.tile([128, 1], F32)
            nc.scalar.dma_start(out=bias_sb[:], in_=bias_d[:])

            from contextlib import nullcontext

            rep_loop = tc.For_i(0, reps, 1) if reps > 1 else nullcontext()
            with rep_loop:
                _emit_body(nc, tc, n_groups, x_d, y_d, wbsb, se_sb, bias_sb,
                           xpool, wfin_pool, wtmp_pool, outp, psump)

    if split_waits:
        _split_multiwait_insts(nc)
    return nc


def _emit_body(nc, tc, n_groups, x_d, y_d, wbsb, se_sb, bias_sb,
               xpool, wfin_pool, wtmp_pool, outp, psump):
    for g in range(n_groups):
        # --- load pre-padded group images: 128-partition DMAs, split
        # into row bands so the first chunks' matmuls start early ------
        xt = xpool.tile([128, HP, WP], BF16)
        for r0, r1 in ((0, 33), (33, 66), (66, 98), (98, HP)):
            nc.sync.dma_start(out=xt[:, r0:r1], in_=x_d[g, :, r0:r1])

        # --- per-sample weight generation -----------------------------
        # wfin[(s,i), t, o] = sum_n se[(s,i),g,n] * wbsb[(s,i),n,t,o]
        wfin = wfin_pool.tile([128, NTAPS, O], F32)
        tmps = []
        for n in range(1, NUM):
            t_ = wtmp_pool.tile([128, NTAPS, O], F32, tag="wtmp")
            nc.scalar.activation(
                t_[:],
                wbsb[:, n],
                mybir.ActivationFunctionType.Identity,
                scale=se_sb[:, g, n : n + 1],
            )
            tmps.append(t_)
        nc.scalar.activation(
            wfin[:],
            wbsb[:, 0],
            mybir.ActivationFunctionType.Identity,
            scale=se_sb[:, g, 0:1],
        )
        for t_ in tmps:
            nc.vector.tensor_add(wfin[:], wfin[:], t_[:])
        # block-diagonal bf16 stationary: wbd[(s,i), t, 32s+o] = wfin,
        # zeros elsewhere (zeroed once per slot; diagonal blocks
        # overwritten each group).
        wbd = wfin_pool.tile([128, NTAPS, 128], BF16, tag="wbd")
        if g < WBD_BUFS:
            nc.gpsimd.memset(wbd[:], 0.0)
        for s in range(GROUP):
            nc.vector.tensor_copy(
                wbd[32 * s : 32 * (s + 1), :, 32 * s : 32 * (s + 1)],
                wfin[32 * s : 32 * (s + 1)],
            )

        # --- conv: tap-major waves of 4 chunks (16 output rows). The
        # stationary tile is reused for 4 consecutive matmuls, amortizing
        # the PE weight-reload (~40ns/matmul on HW when it changes every
        # matmul). 4 PSUM banks accumulate per wave; 8 banks total let
        # wave w+1 accumulate while wave w drains. ---------------------
        for wv in range(N_CHUNKS // 4):
            pss = []
            for j in range(4):
                ps_j = psump.tile([128, ROWS_PER_CHUNK * W], F32, tag="psw")
                pss.append(ps_j)
            for tap in range(NTAPS):
                kh, kw = divmod(tap, KK)
                for j in range(4):
                    h0 = (wv * 4 + j) * ROWS_PER_CHUNK
                    nc.tensor.matmul(
                        pss[j][:],
                        wbd[:, tap, :],
                        xt[:, h0 + kh : h0 + kh + ROWS_PER_CHUNK, kw : kw + W],
                        start=(tap == 0),
                        stop=(tap == NTAPS - 1),
                    )
            # drain PSUM -> bf16 SBUF with fused bias, alternating ACT/DVE
            # so neither queue serializes the epilogue; one DMA per two
            # chunks so each store writes 2KB contiguous per partition.
            for j in (0, 2):
                h0 = (wv * 4 + j) * ROWS_PER_CHUNK
                ob = outp.tile([128, 2, ROWS_PER_CHUNK * W], BF16, tag="ob")
                nc.scalar.activation(
                    ob[:, 0],
                    pss[j][:],
                    mybir.ActivationFunctionType.Identity,
                    bias=bias_sb[:, 0:1],
                )
                nc.vector.tensor_scalar_add(ob[:, 1], pss[j + 1][:], bias_sb[:, 0:1])
                store_eng = nc.scalar if j == 0 else nc.sync
                store_eng.dma_start(
                    out=y_d[
                        g * GROUP : (g + 1) * GROUP,
                        :,
                        h0 : h0 + 2 * ROWS_PER_CHUNK,
                        :,
                    ],
                    in_=ob[:],
                )


def _host_prep(inputs, inputs_se, weight, bias):
    """Shard + relayout the inputs for the 8 per-core programs."""
    inputs_se = np.asarray(inputs_se, dtype=np.float32)
    weight = np.asarray(weight, dtype=np.float32)
    bias = np.asarray(bias, dtype=np.float32)

    # pre-padded bf16 images
    x_pad = np.zeros((B, C, HP, WP), dtype=NP_BF16)
    x_pad[:, :, 1 : H + 1, 1 : W + 1] = np.asarray(inputs, dtype=np.float32)

    # wb[32s+i, n, t*32+o] = weight[o*288 + i*9 + t, n], tiled over s
    wb = weight.reshape(O, C, NTAPS, NUM)          # [o, i, t, n]
    wb = wb.transpose(1, 3, 2, 0).reshape(C, NUM, NTAPS * O)
    wb = np.ascontiguousarray(np.tile(wb, (GROUP, 1, 1)))  # [128, 8, 288]
    bias_rep = np.ascontiguousarray(np.tile(bias, GROUP)[:, None])  # [128, 1]

    in_maps = []
    for core in range(N_CORES):
        b0 = core * B_LOCAL
        se_loc = inputs_se[b0 : b0 + B_LOCAL]      # [16, 8]
        # se[32*s+i, g, n] = se_loc[g*4+s, n]
        se_exp = np.repeat(
            se_loc.reshape(N_GROUPS, GROUP, NUM).transpose(1, 0, 2), 32, axis=0
        )  # [4*32, g, n] with (s, i) partition order
        in_maps.append(
            {
                "x": x_pad[b0 : b0 + B_LOCAL].reshape(
                    N_GROUPS, GROUP * C, HP, WP
                ),
                "wb": wb,
                "se": np.ascontiguousarray(se_exp, dtype=np.float32),
                "bias": bias_rep,
            }
        )
    return in_maps


_NC_CACHE = {}


def kernel(inputs, inputs_se, weight, bias):
    if "nc" not in _NC_CACHE:
        _NC_CACHE["nc"] = build_program()
    nc = _NC_CACHE["nc"]
    in_maps = _host_prep(inputs, inputs_se, weight, bias)
    res = run_bass_kernel_spmd(nc, in_maps, list(range(N_CORES)))
    out = np.concatenate(
        [np.asarray(res.results[i]["y"]) for i in range(N_CORES)], axis=0
    )
    return out.astype(np.float32)
